# revision 27
# baseline (speedup 1.0000x reference)
"""AttentionBlock (GroupNorm + qkv 1x1 + 4-head attention over T=4096 + proj 1x1
+ residual) for b=2, c=256, H=W=64 on 8 NeuronCores.

Sharding: one (batch, head) pair per core (b*nh = 8 = n_cores).

v2 design (PE row-tiling + 2-engine fp8 exp):
  - x arrives bf16 (host casts; the f32 residual is added on host), DMA'd in
    half-tile chunks so bn_stats overlaps the transfer. GroupNorm group
    reduce/broadcast via one-hot matmuls. A dummy-matmul burst gated on the
    x arrival holds the PE HAM clock gate at 2.4 GHz into the main phase
    (the gate needs ~3.4us of sustained activity and re-throttles after a
    ~3.4us idle window).
  - q is emitted DUPLICATED into both partition halves ([wq|wq] weights) and
    k is emitted even/odd-block split (rows 0:64 = even key blocks, 64:128 =
    odd) so the K=64 score matmuls run as 2x row-tiled pairs: two key blocks
    per ~216ns -- 2x the v1 score throughput. tile_position is inferred by
    bass from the AP base partitions.
  - exp of every score pair produces fp8 p (uniform 2^-1 scale): scalar
    engine AF.Exp (scale=1/8, bias=-ln2) and DVE Schraudolph-to-fp8-bits
    (uint8 = clip(round(1.4427*s + 48)) bitcast fp8e4m3; f32->uint8
    conversion saturates at 0 so underflow is exactly +0.0). The per-pair
    engine split is tuned to balance the two queues.
  - h += vT.T @ p via fp8 DoubleRow pairs (1024 cols @ 0.5 cyc/col); vT
    carries a ones column so ps_h row 64 is the softmax denominator.
  - unnormalized proj partials stream out per 512-col chunk in bf16; host
    gather applies 1/rowsum, W_p@b_v, proj_b and the f32 residual.
  - software pipelining: each t-chunk's last DoubleRow batch is flushed
    early in the next t-chunk and epilogue(i-1) is emitted after body(i),
    so neither the exp tail nor the proj/copy burst serializes the
    boundary. Steady state is scalar-exp-bound at ~11.5us per t-chunk.
"""

import sys
import types

import numpy as np
import ml_dtypes

# ---------------------------------------------------------------------------
# Environment shims (axon container): NTFF profile hook + no artifact upload.
# ---------------------------------------------------------------------------


def _install_shims():
    if "antenv.axon_hooks" not in sys.modules:
        mod = types.ModuleType("antenv.axon_hooks")
        _hook = [None]
        mod.set_axon_ntff_profile_hook = lambda h: _hook.__setitem__(0, h)
        mod.get_axon_ntff_profile_hook = lambda: _hook[0]
        sys.modules["antenv.axon_hooks"] = mod
        try:
            import antenv

            antenv.axon_hooks = mod
            from trn_agent_boot.trn_boot import _ntff_profile_via_ctypes

            mod.set_axon_ntff_profile_hook(
                _ntff_profile_via_ctypes("/opt/axon/libaxon_pjrt.so")
            )
        except Exception:
            pass
    import concourse.bass_utils as bass_utils

    bass_utils.upload_artifacts = lambda d: d


_install_shims()

import concourse.bass as bass
import concourse.mybir as mybir
import concourse.tile as tile
from concourse.bass_utils import run_bass_kernel_spmd

F32 = mybir.dt.float32
BF16 = mybir.dt.bfloat16
U8 = mybir.dt.uint8
FP8 = mybir.dt.float8e4
AF = mybir.ActivationFunctionType
ALU = mybir.AluOpType
DRMODE = mybir.MatmulPerfMode.DoubleRow

B, C, HW, T = 2, 256, 64, 4096
NH, CH = 4, 64  # heads, channels per head
NG, GS = 32, 8  # groups, channels per group
EPS = 1e-5
N_CORES = 8
TC = 512  # t-chunk width
N_TCHUNKS = T // TC  # 8
N_PAIRS = 16  # key-block pairs per t-chunk (32 blocks of 128)
LN2 = 0.6931471805599453
# Schraudolph fp8e4m3 bits: byte = round(1.4427*s_raw + 48) => exp(s_raw/8)/2
SCH_A = 1.4426950408889634
SCH_B = 48.0
# exp engine per pair: 'S' scalar AF.Exp, 'V' DVE Schraudolph (9S/7V --
# scalar's exp is ~1.09us vs DVE's ~1.37us per pair; V leads so the one
# exp-table reload overlaps DVE work at tchunk-0 start)
EXP_ENG = list("SVSSVSSVSVSSVSVS")


# ---------------------------------------------------------------------------
# BIR wait legalization: this container's walrus accepts at most ONE sync wait
# per instruction (two for EventSemaphore); hoist excess waits onto inserted
# EventSemaphores on the same engine.
# ---------------------------------------------------------------------------


def _legalize_bir_waits(bir_bytes: bytes) -> bytes:
    import json

    m = json.loads(bir_bytes)
    changed = False
    for fn in m["functions"]:
        for blk in fn["blocks"]:
            new_insts = []
            for inst in blk["instructions"]:
                si = inst.get("sync_info")
                waits = list(si.get("on_wait") or []) if si else []
                cap = 2 if inst.get("opcode") == "EventSemaphore" else 1
                if len(waits) > cap:
                    changed = True
                    keep = waits[-cap:]
                    extra = waits[:-cap]
                    idx = 0
                    while extra:
                        chunk, extra = extra[:2], extra[2:]
                        es = {
                            "name": f"{inst['name']}_ws{idx}",
                            "engine": inst["engine"],
                            "opcode": "EventSemaphore",
                            "ins": [],
                            "outs": [],
                            "sync_info": {"on_wait": chunk, "on_update": []},
                        }
                        if "debug" in inst:
                            es["debug"] = inst["debug"]
                        new_insts.append(es)
                        idx += 1
                    si["on_wait"] = keep
                new_insts.append(inst)
            blk["instructions"] = new_insts
    return json.dumps(m).encode() if changed else bir_bytes


# ---------------------------------------------------------------------------
# Device program (identical on all 8 cores; inputs differ per core)
# ---------------------------------------------------------------------------


def build_nc():
    nc = bass.Bass()

    x_in = nc.dram_tensor("x", [C, T], BF16, kind="ExternalInput")
    # [wq|wq] and [wk|wk] duplicated weights, per 128-channel half
    wqq_in = nc.dram_tensor("wqq", [C, 128], BF16, kind="ExternalInput")
    wkk_in = nc.dram_tensor("wkk", [C, 128], BF16, kind="ExternalInput")
    wvT_in = nc.dram_tensor("wvT", [C, CH], BF16, kind="ExternalInput")
    bqq_in = nc.dram_tensor("bqq", [128, 1], F32, kind="ExternalInput")
    bkk_in = nc.dram_tensor("bkk", [128, 1], F32, kind="ExternalInput")
    # proj weights zero-padded to K=128 (rows CH..127 are zero)
    wpT_in = nc.dram_tensor("wpT", [128, C], BF16, kind="ExternalInput")
    oh_in = nc.dram_tensor("oh", [128, 16], F32, kind="ExternalInput")
    ohT_in = nc.dram_tensor("ohT", [16, 128], F32, kind="ExternalInput")
    gnw_in = nc.dram_tensor("gnw", [C, 1], F32, kind="ExternalInput")
    gnb_in = nc.dram_tensor("gnb", [C, 1], F32, kind="ExternalInput")
    out = nc.dram_tensor("out", [C, T], BF16, kind="ExternalOutput")
    # softmax denominators (host divides during the gather)
    rsum_out = nc.dram_tensor("rsum", [1, T], BF16, kind="ExternalOutput")

    with tile.TileContext(nc) as tc:
        with (
            tc.tile_pool(name="const", bufs=1) as const,
            tc.tile_pool(name="xp", bufs=2) as xp,
            tc.tile_pool(name="xnp", bufs=2) as xnp,
            tc.tile_pool(name="qk", bufs=1) as qkp,
            tc.tile_pool(name="gn", bufs=2) as gn,
            tc.tile_pool(name="pp", bufs=8) as ppool,
            tc.tile_pool(name="op", bufs=4) as op,
            tc.tile_pool(name="ps", bufs=3, space="PSUM") as ps,
            tc.tile_pool(name="ph", bufs=2, space="PSUM") as ph,
        ):
            # early exp-table prefetch: the first scalar ACTIVATE loads the
            # exp set into table slot 0 (Sqrt later takes slot 1), so no
            # table load lands on the attention critical path
            warmup_c = const.tile([2, 1], F32, tag="warmup_c")
            nc.vector.memset(warmup_c, 0.0)
            nc.scalar.activation(out=warmup_c, in_=warmup_c, func=AF.Exp)

            # PE HAM warm-up fodder (zeros): the clock gate needs ~3.4us of
            # sustained matmul activity to release 2.4 GHz
            warm_w = const.tile([128, 512], BF16, tag="warm_w")
            nc.vector.memset(warm_w, 0.0)

            # ---- x load first (bf16, halves per tile so stats can overlap) ----
            x_tiles = []
            for i in range(2):
                x_t = xp.tile([128, T], BF16, tag="x", name=f"x{i}")
                for half in range(2):
                    hsl = slice(half * (T // 2), (half + 1) * (T // 2))
                    nc.sync.dma_start(
                        out=x_t[:, hsl], in_=x_in[i * 128 : (i + 1) * 128, hsl]
                    )
                x_tiles.append(x_t)

            # warm-up burst: dummy matmuls gated on x tile 1's arrival so the
            # PE clock is warm when the first score pairs issue (~10us later)
            wps = ps.tile([128, 1024], F32, tag="ps", name="warm_ps")
            for i in range(20):
                nc.tensor.matmul(
                    wps[:, 0:512],
                    lhsT=warm_w[:, 0:128],
                    rhs=x_tiles[1][:, 0:512],
                    start=True, stop=True,
                )

            # ---- load constants/weights ----
            def load_const(name, src, shape, dtype):
                t = const.tile(shape, dtype, tag=name)
                nc.gpsimd.dma_start(out=t, in_=src[:, :])
                return t

            wqq = [
                const.tile([128, 128], BF16, tag=f"wqq{k}", name=f"wqq{k}")
                for k in range(2)
            ]
            wkk = [
                const.tile([128, 128], BF16, tag=f"wkk{k}", name=f"wkk{k}")
                for k in range(2)
            ]
            wv = [
                const.tile([128, CH], BF16, tag=f"wv{k}", name=f"wv{k}")
                for k in range(2)
            ]
            for k in range(2):
                sl = slice(k * 128, (k + 1) * 128)
                nc.gpsimd.dma_start(out=wqq[k], in_=wqq_in[sl, :])
                nc.gpsimd.dma_start(out=wkk[k], in_=wkk_in[sl, :])
                nc.gpsimd.dma_start(out=wv[k], in_=wvT_in[sl, :])

            bqq_sb = load_const("bqq", bqq_in, [128, 1], F32)
            bkk_sb = load_const("bkk", bkk_in, [128, 1], F32)
            wpT_sb = load_const("wpT", wpT_in, [128, C], BF16)
            oh_sb = load_const("oh", oh_in, [128, 16], F32)
            ohT_sb = load_const("ohT", ohT_in, [16, 128], F32)

            gnw_t = [
                const.tile([128, 1], F32, tag=f"gnw{i}", name=f"gnw{i}")
                for i in range(2)
            ]
            gnb_t = [
                const.tile([128, 1], F32, tag=f"gnb{i}", name=f"gnb{i}")
                for i in range(2)
            ]
            for i in range(2):
                nc.gpsimd.dma_start(out=gnw_t[i], in_=gnw_in[i * 128 : (i + 1) * 128, :])
                nc.gpsimd.dma_start(out=gnb_t[i], in_=gnb_in[i * 128 : (i + 1) * 128, :])

            eps_t = const.tile([16, 1], F32, tag="eps")
            nc.vector.memset(eps_t, EPS)
            nln2_t = const.tile([128, 1], F32, tag="nln2")
            nc.vector.memset(nln2_t, -LN2)

            # persistent hu tiles: rows 65..127 zeroed once (row 64 = denom,
            # masked by wpT's zero rows)
            hu_t = []
            for i in range(2):
                t = const.tile([128, TC], BF16, tag=f"hu{i}", name=f"hu{i}")
                nc.vector.memset(t[CH:128, :], 0.0)
                hu_t.append(t)

            # vT with ones columns (fp8): [128 keys, 32 blocks, 80] -- cols
            # 0:64 v channels, col 64 ones (denominator row of ps_h)
            vT = qkp.tile([128, 32 * 80], FP8, tag="vT")
            nc.gpsimd.memset(vT, 1.0)
            vT_view = vT.rearrange("p (b c) -> p b c", c=80)

            # ---- GroupNorm stats -> per-channel affine (a_ch, b_ch) ----
            ab_ch = []  # per tile: (a, b)
            for i in range(2):
                x_t = x_tiles[i]
                xv = x_t.rearrange("p (n f) -> p n f", f=512)
                stats = gn.tile([128, 8, 6], F32, tag="stats")
                for j in range(8):
                    nc.vector.bn_stats(out=stats[:, j, :], in_=xv[:, j, :])
                    if j % 2 == 1:
                        wph = ph.tile([6, 16], F32, tag="ph", name=f"wt{i}_{j}")
                        nc.tensor.matmul(
                            wph[0:6, 0:6], lhsT=stats[:, j, :], rhs=stats[:, j, :],
                            start=True, stop=True,
                        )
                mv = gn.tile([128, 2], F32, tag="mv")
                nc.vector.bn_aggr(out=mv, in_=stats)

                # mq = [mean, var + mean^2] per channel
                mq = gn.tile([128, 2], F32, tag="mq")
                nc.vector.tensor_copy(out=mq[:, 0:1], in_=mv[:, 0:1])
                m2 = gn.tile([128, 1], F32, tag="m2")
                nc.vector.tensor_tensor(
                    out=m2, in0=mv[:, 0:1], in1=mv[:, 0:1], op=ALU.mult
                )
                nc.vector.tensor_tensor(
                    out=mq[:, 1:2], in0=mv[:, 1:2], in1=m2, op=ALU.add
                )

                # group reduce: [16, 2] = oh.T @ mq   (oh entries are 1/8)
                ps_g = ph.tile([16, 2], F32, tag="ph")
                nc.tensor.matmul(ps_g, lhsT=oh_sb, rhs=mq, start=True, stop=True)
                gstats = gn.tile([16, 2], F32, tag="gstats")
                nc.vector.tensor_copy(out=gstats, in_=ps_g)

                gm2 = gn.tile([16, 1], F32, tag="gm2")
                nc.vector.tensor_tensor(
                    out=gm2, in0=gstats[:, 0:1], in1=gstats[:, 0:1], op=ALU.mult
                )
                gvar = gn.tile([16, 1], F32, tag="gvar")
                nc.vector.tensor_tensor(
                    out=gvar, in0=gstats[:, 1:2], in1=gm2, op=ALU.subtract
                )
                sq = gn.tile([16, 1], F32, tag="sq")
                nc.scalar.activation(out=sq, in_=gvar, func=AF.Sqrt, bias=eps_t)
                grstd = gn.tile([16, 1], F32, tag="grstd")
                nc.vector.reciprocal(out=grstd, in_=sq)
                gmr = gn.tile([16, 2], F32, tag="gmr")
                nc.vector.tensor_copy(out=gmr[:, 0:1], in_=gstats[:, 0:1])
                nc.vector.tensor_copy(out=gmr[:, 1:2], in_=grstd)

                # broadcast back to channels: [128, 2] = ohT.T @ gmr
                ps_bc = ph.tile([128, 2], F32, tag="ph")
                nc.tensor.matmul(ps_bc, lhsT=ohT_sb, rhs=gmr, start=True, stop=True)

                a_ch = gn.tile([128, 1], F32, tag="a_ch")
                nc.vector.tensor_tensor(
                    out=a_ch, in0=ps_bc[:, 1:2], in1=gnw_t[i], op=ALU.mult
                )
                t1 = gn.tile([128, 1], F32, tag="t1")
                nc.vector.tensor_tensor(
                    out=t1, in0=ps_bc[:, 0:1], in1=a_ch, op=ALU.mult
                )
                b_ch = gn.tile([128, 1], F32, tag="b_ch")
                nc.vector.tensor_tensor(
                    out=b_ch, in0=gnb_t[i], in1=t1, op=ALU.subtract
                )
                ab_ch.append((a_ch, b_ch))

            # ---- xn tiles (bf16), emitted per 1024-col chunk on DVE ----
            xn_tiles = []
            for i in range(2):
                xn_t = xnp.tile([128, T], BF16, tag="xn", name=f"xn{i}")
                xn_tiles.append(xn_t)

            def emit_xn_chunk(c):
                csl = slice(c * 1024, (c + 1) * 1024)
                for i in range(2):
                    a_ch, b_ch = ab_ch[i]
                    with nc.allow_low_precision(reason="bf16 xn"):
                        nc.vector.tensor_scalar(
                            out=xn_tiles[i][:, csl],
                            in0=x_tiles[i][:, csl],
                            scalar1=a_ch,
                            scalar2=b_ch,
                            op0=ALU.mult,
                            op1=ALU.add,
                        )

            # ---- q2 (dup) / k2 (even-odd) / vT emission ----
            q2 = qkp.tile([128, T], BF16, tag="q2")
            k2 = qkp.tile([128, 2048], BF16, tag="k2")

            def emit_q_chunk(c):
                # two [128,512] halves through the ph pool so the score ring
                # keeps all three slots for pairs
                for h in range(2):
                    xsl = slice(c * 1024 + h * 512, c * 1024 + (h + 1) * 512)
                    psq = ph.tile([128, 512], F32, tag="ph", name=f"psq{c}_{h}")
                    for ki in range(2):
                        nc.tensor.matmul(
                            psq,
                            lhsT=wqq[ki],
                            rhs=xn_tiles[ki][:, xsl],
                            start=(ki == 0),
                            stop=(ki == 1),
                        )
                    with nc.allow_low_precision(reason="bf16 q"):
                        if c % 2 == 0:
                            nc.scalar.activation(
                                out=q2[:, xsl], in_=psq, func=AF.Identity,
                                bias=bqq_sb,
                            )
                        else:
                            nc.vector.tensor_scalar(
                                out=q2[:, xsl], in0=psq, scalar1=bqq_sb,
                                scalar2=None, op0=ALU.add,
                            )

            def emit_k_chunk(c):
                # chunk c covers blocks 8c..8c+7 -> k2 cols [c*512,(c+1)*512);
                # two [128,512] halves through the ph pool (ring stays free)
                for h in range(2):
                    xsl = slice(c * 1024 + h * 512, c * 1024 + (h + 1) * 512)
                    psk = ph.tile([128, 512], F32, tag="ph", name=f"psk{c}_{h}")
                    for ki in range(2):
                        nc.tensor.matmul(
                            psk,
                            lhsT=wkk[ki],
                            rhs=xn_tiles[ki][:, xsl],
                            start=(ki == 0),
                            stop=(ki == 1),
                        )
                    ksl = slice(c * 512 + h * 256, c * 512 + (h + 1) * 256)
                    pv = psk.rearrange("p (m t c) -> p m t c", t=2, c=128)
                    with nc.allow_low_precision(reason="bf16 k"):
                        # even blocks -> rows 0:64, odd -> rows 64:128
                        nc.vector.tensor_scalar(
                            out=k2[0:64, ksl].rearrange("p (m c) -> p m c", c=128),
                            in0=pv[0:64, :, 0:1, :].rearrange("p m t c -> p m (t c)"),
                            scalar1=bkk_sb[0:64],
                            scalar2=None,
                            op0=ALU.add,
                        )
                        nc.vector.tensor_scalar(
                            out=k2[64:128, ksl].rearrange("p (m c) -> p m c", c=128),
                            in0=pv[64:128, :, 1:2, :].rearrange("p m t c -> p m (t c)"),
                            scalar1=bkk_sb[64:128],
                            scalar2=None,
                            op0=ALU.add,
                        )

            def emit_v_chunk(c):
                # blocks 8c..8c+7
                psv = ph.tile([128, 512], F32, tag="ph", name=f"psv{c}")
                for j in range(8):
                    sblk = c * 8 + j
                    sl = slice(j * 64, (j + 1) * 64)
                    for ki in range(2):
                        nc.tensor.matmul(
                            psv[:, sl],
                            lhsT=xn_tiles[ki][:, sblk * 128 : (sblk + 1) * 128],
                            rhs=wv[ki],
                            start=(ki == 0),
                            stop=(ki == 1),
                        )
                with nc.allow_low_precision(reason="fp8 v"):
                    nc.scalar.copy(
                        out=vT_view[:, c * 8 : (c + 1) * 8, 0:64],
                        in_=psv.rearrange("p (b c) -> p b c", c=64),
                    )

            emit_xn_chunk(0)
            emit_q_chunk(0)
            emit_k_chunk(0)
            emit_v_chunk(0)
            for c in range(1, 4):
                emit_xn_chunk(c)

            # ---- attention + proj, streamed per t-chunk ----
            def body(tci, carry=None):
                tsl = slice(tci * TC, (tci + 1) * TC)
                ps_h = ph.tile([65, TC], F32, tag="ph", name=f"ps_h{tci}")
                pend_dr = []

                def flush_drs():
                    for pr, m in pend_dr:
                        nc.tensor.matmul(
                            ps_h,
                            lhsT=vT_view[:, 2 * m : 2 * m + 2, 0:65],
                            rhs=pr,
                            start=(m == 0),
                            stop=(m == N_PAIRS - 1),
                            perf_mode=DRMODE,
                        )
                    pend_dr.clear()

                def flush_carry():
                    if carry is None:
                        return
                    c_ps_h, c_pend = carry
                    for pr, m in c_pend:
                        nc.tensor.matmul(
                            c_ps_h,
                            lhsT=vT_view[:, 2 * m : 2 * m + 2, 0:65],
                            rhs=pr,
                            start=(m == 0),
                            stop=(m == N_PAIRS - 1),
                            perf_mode=DRMODE,
                        )
                    c_pend.clear()

                for m in range(N_PAIRS):
                    # JIT emission: k/v chunks during tchunk 0, q chunks on
                    # odd tchunks (q chunk c gates tchunk 2c)
                    if tci == 0 and m in (4, 8, 12):
                        flush_drs()
                        emit_k_chunk(m // 4)
                    if tci == 0 and m in (6, 10, 14):
                        emit_v_chunk(m // 4)
                    if tci in (1, 3, 5) and m == 4:
                        flush_drs()
                        emit_q_chunk((tci + 1) // 2)
                    ps_s = ps.tile([128, 1024], F32, tag="ps", name=f"ps_s{tci}_{m}")
                    # row-tiled pair: even block (rows 0:64) + odd (64:128)
                    nc.tensor.matmul(
                        ps_s[:, 0:512],
                        lhsT=k2[0:64, m * 128 : (m + 1) * 128],
                        rhs=q2[0:64, tsl],
                        start=True,
                        stop=True,
                    )
                    nc.tensor.matmul(
                        ps_s[:, 512:1024],
                        lhsT=k2[64:128, m * 128 : (m + 1) * 128],
                        rhs=q2[64:128, tsl],
                        start=True,
                        stop=True,
                    )
                    p_t = ppool.tile([128, 1024], FP8, tag="p", name=f"p{tci}_{m}")
                    with nc.allow_low_precision(reason="fp8 p"):
                        if EXP_ENG[m] == "S":
                            nc.scalar.activation(
                                out=p_t,
                                in_=ps_s,
                                func=AF.Exp,
                                scale=0.125,
                                bias=nln2_t,
                            )
                        else:
                            nc.vector.tensor_scalar(
                                out=p_t.bitcast(U8),
                                in0=ps_s,
                                scalar1=SCH_A,
                                scalar2=SCH_B,
                                op0=ALU.mult,
                                op1=ALU.add,
                            )
                    pend_dr.append((p_t.rearrange("p (b c) -> p b c", c=TC), m))
                    if m % 4 == 3 and m < N_PAIRS - 1:
                        flush_drs()
                return ps_h, pend_dr

            def epilogue(tci, ps_h):
                tsl = slice(tci * TC, (tci + 1) * TC)
                hu = hu_t[tci % 2]
                with nc.allow_low_precision(reason="bf16 h"):
                    nc.vector.tensor_copy(out=hu[0:65, :], in_=ps_h[0:65, :])
                nc.sync.dma_start(out=rsum_out[0:1, tsl], in_=hu[64:65, :])
                for mi in range(2):
                    pp_ps = ph.tile([128, TC], F32, tag="ph", name=f"pp{tci}_{mi}")
                    nc.tensor.matmul(
                        pp_ps,
                        lhsT=wpT_sb[:, mi * 128 : (mi + 1) * 128],
                        rhs=hu,
                        start=True,
                        stop=True,
                    )
                    o_t = op.tile([128, TC], BF16, tag="o", name=f"o{tci}_{mi}")
                    with nc.allow_low_precision(reason="bf16 out"):
                        nc.vector.tensor_copy(out=o_t, in_=pp_ps)
                    nc.sync.dma_start(
                        out=out[mi * 128 : (mi + 1) * 128, tsl], in_=o_t
                    )

            # software pipeline: the last DR batch of chunk i is flushed
            # early in chunk i+1, and epilogue(i-1) is emitted after body(i)
            prev = None
            for tci in range(N_TCHUNKS):
                cur = body(tci, carry=prev)
                if prev is not None:
                    epilogue(tci - 1, prev[0])
                prev = cur
            # drain the final tchunk
            ps_h_f, pend_f = prev
            for pr, m in pend_f:
                nc.tensor.matmul(
                    ps_h_f,
                    lhsT=vT_view[:, 2 * m : 2 * m + 2, 0:65],
                    rhs=pr,
                    start=(m == 0),
                    stop=(m == N_PAIRS - 1),
                    perf_mode=DRMODE,
                )
            epilogue(N_TCHUNKS - 1, ps_h_f)

    orig = nc.to_json_bytes
    nc.to_json_bytes = lambda *a, **k: _legalize_bir_waits(orig(*a, **k))
    return nc


_NC = None


def _get_nc():
    global _NC
    if _NC is None:
        _NC = build_nc()
    return _NC


def _make_in_maps(inputs):
    x = np.asarray(inputs["x"], dtype=np.float32)
    gn_w = np.asarray(inputs["gn_w"], dtype=np.float32)
    gn_b = np.asarray(inputs["gn_b"], dtype=np.float32)
    qkv_w = np.asarray(inputs["qkv_w"], dtype=np.float32)
    qkv_b = np.asarray(inputs["qkv_b"], dtype=np.float32)
    proj_w = np.asarray(inputs["proj_w"], dtype=np.float32)

    xs = x.reshape(B, C, T).astype(ml_dtypes.bfloat16)
    oh = np.kron(np.eye(16, dtype=np.float32), np.full((8, 1), 0.125, np.float32))
    ohT = np.ascontiguousarray(oh.T) * 8.0  # plain one-hot [16, 128]
    gnw = gn_w.reshape(C, 1)
    gnb = gn_b.reshape(C, 1)

    in_maps = []
    for core in range(N_CORES):
        b, h = divmod(core, NH)
        # reference reshapes (b, 3c, T) -> (b*nh, 3*ch, T) then splits dim 1,
        # so head h takes qkv rows [3*ch*h : 3*ch*(h+1)] as [q | k | v]
        base = 3 * CH * h
        qsl = slice(base, base + CH)
        ksl = slice(base + CH, base + 2 * CH)
        vsl = slice(base + 2 * CH, base + 3 * CH)
        wqT = np.ascontiguousarray(qkv_w[qsl, :].T)  # [C, CH]
        wkT = np.ascontiguousarray(qkv_w[ksl, :].T)
        wqq = np.concatenate([wqT, wqT], axis=1).astype(ml_dtypes.bfloat16)
        wkk = np.concatenate([wkT, wkT], axis=1).astype(ml_dtypes.bfloat16)
        wvT = np.ascontiguousarray(qkv_w[vsl, :].T).astype(ml_dtypes.bfloat16)
        bq = qkv_b[qsl].astype(np.float32)
        bk = qkv_b[ksl].astype(np.float32)
        bqq = np.concatenate([bq, bq]).reshape(128, 1)
        bkk = np.concatenate([bk, bk]).reshape(128, 1)
        # after attention, head h occupies channels [ch*h : ch*(h+1)]
        wpT = np.zeros((128, C), ml_dtypes.bfloat16)
        wpT[0:CH] = proj_w[:, h * CH : (h + 1) * CH].T.astype(ml_dtypes.bfloat16)
        in_maps.append(
            {
                "x": np.ascontiguousarray(xs[b]),
                "wqq": wqq,
                "wkk": wkk,
                "wvT": wvT,
                "bqq": bqq,
                "bkk": bkk,
                "wpT": wpT,
                "oh": oh,
                "ohT": ohT,
                "gnw": gnw,
                "gnb": gnb,
            }
        )
    return in_maps


def _combine(inputs, results):
    x = np.asarray(inputs["x"], dtype=np.float32)
    proj_b = np.asarray(inputs["proj_b"], dtype=np.float32)
    qkv_b = np.asarray(inputs["qkv_b"], dtype=np.float32)
    proj_w = np.asarray(inputs["proj_w"], dtype=np.float32)
    xs = x.reshape(B, C, T)
    out = np.empty((B, C, T), np.float32)
    for b in range(B):
        acc = xs[b] + proj_b[:, None]
        for h in range(NH):
            r = results[b * NH + h]
            # v's bias bv contributes bv (x) rowsum to the unnormalized h;
            # after proj and the rowsum division it is the constant vector
            # proj_w[:, head] @ bv -- folded here instead of on device
            bv = qkv_b[3 * CH * h + 2 * CH : 3 * CH * (h + 1)]
            wpbv = proj_w[:, h * CH : (h + 1) * CH] @ bv
            acc = (
                acc
                + r["out"].astype(np.float32)
                * (1.0 / r["rsum"][0].astype(np.float32))[None, :]
                + wpbv[:, None]
            )
        out[b] = acc
    return out.reshape(B, C, HW, HW)


def _run(inputs, trace=False, trace_kwargs=None):
    nc = _get_nc()
    in_maps = _make_in_maps(inputs)
    res = run_bass_kernel_spmd(
        nc,
        in_maps,
        core_ids=list(range(N_CORES)),
        trace=trace,
        **(trace_kwargs or {}),
    )
    return _combine(inputs, res.results), res


def kernel(**inputs) -> np.ndarray:
    out, _ = _run(inputs, trace=False)
    return out


# revision 28
# speedup vs baseline: 1.1041x; 1.1041x over previous
"""AttentionBlock (GroupNorm + qkv 1x1 + 4-head attention over T=4096 + proj 1x1
+ residual) for b=2, c=256, H=W=64 on 8 NeuronCores.

Sharding: one (batch, head) pair per core (b*nh = 8 = n_cores).

v2 design (PE row-tiling + 2-engine fp8 exp):
  - x arrives bf16 (host casts; the f32 residual is added on host), DMA'd in
    half-tile chunks so bn_stats overlaps the transfer. GroupNorm group
    reduce/broadcast via one-hot matmuls. A dummy-matmul burst gated on the
    x arrival holds the PE HAM clock gate at 2.4 GHz into the main phase
    (the gate needs ~3.4us of sustained activity and re-throttles after a
    ~3.4us idle window).
  - q is emitted DUPLICATED into both partition halves ([wq|wq] weights) and
    k is emitted even/odd-block split (rows 0:64 = even key blocks, 64:128 =
    odd) so the K=64 score matmuls run as 2x row-tiled pairs: two key blocks
    per ~216ns -- 2x the v1 score throughput. tile_position is inferred by
    bass from the AP base partitions.
  - exp of every score pair produces fp8 p (uniform 2^-1 scale): scalar
    engine AF.Exp (scale=1/8, bias=-ln2) and DVE Schraudolph-to-fp8-bits
    (uint8 = clip(round(1.4427*s + 48)) bitcast fp8e4m3; f32->uint8
    conversion saturates at 0 so underflow is exactly +0.0). The per-pair
    engine split is tuned to balance the two queues.
  - h += vT.T @ p via fp8 DoubleRow pairs (1024 cols @ 0.5 cyc/col); vT
    carries a ones column so ps_h row 64 is the softmax denominator.
  - unnormalized proj partials stream out per 512-col chunk in bf16; host
    gather applies 1/rowsum, W_p@b_v, proj_b and the f32 residual.
  - software pipelining: each t-chunk's last DoubleRow batch is flushed
    early in the next t-chunk and epilogue(i-1) is emitted after body(i),
    so neither the exp tail nor the proj/copy burst serializes the
    boundary. Steady state is scalar-exp-bound at ~11.5us per t-chunk.
"""

import sys
import types

import numpy as np
import ml_dtypes

# ---------------------------------------------------------------------------
# Environment shims (axon container): NTFF profile hook + no artifact upload.
# ---------------------------------------------------------------------------


def _install_shims():
    if "antenv.axon_hooks" not in sys.modules:
        mod = types.ModuleType("antenv.axon_hooks")
        _hook = [None]
        mod.set_axon_ntff_profile_hook = lambda h: _hook.__setitem__(0, h)
        mod.get_axon_ntff_profile_hook = lambda: _hook[0]
        sys.modules["antenv.axon_hooks"] = mod
        try:
            import antenv

            antenv.axon_hooks = mod
            from trn_agent_boot.trn_boot import _ntff_profile_via_ctypes

            mod.set_axon_ntff_profile_hook(
                _ntff_profile_via_ctypes("/opt/axon/libaxon_pjrt.so")
            )
        except Exception:
            pass
    import concourse.bass_utils as bass_utils

    bass_utils.upload_artifacts = lambda d: d


_install_shims()

import concourse.bass as bass
import concourse.mybir as mybir
import concourse.tile as tile
from concourse.bass_utils import run_bass_kernel_spmd

F32 = mybir.dt.float32
BF16 = mybir.dt.bfloat16
U8 = mybir.dt.uint8
FP8 = mybir.dt.float8e4
AF = mybir.ActivationFunctionType
ALU = mybir.AluOpType
DRMODE = mybir.MatmulPerfMode.DoubleRow

B, C, HW, T = 2, 256, 64, 4096
NH, CH = 4, 64  # heads, channels per head
NG, GS = 32, 8  # groups, channels per group
EPS = 1e-5
N_CORES = 8
TC = 512  # t-chunk width
N_TCHUNKS = T // TC  # 8
N_PAIRS = 16  # key-block pairs per t-chunk (32 blocks of 128)
LN2 = 0.6931471805599453
# Schraudolph fp8e4m3 bits: byte = round(1.4427*s_raw + 48) => exp(s_raw/8)/2
SCH_A = 1.4426950408889634
SCH_B = 48.0
# exp engine per pair: 'S' scalar AF.Exp, 'V' DVE Schraudolph (9S/7V --
# scalar's exp is ~1.09us vs DVE's ~1.37us per pair; V leads so the one
# exp-table reload overlaps DVE work at tchunk-0 start)
EXP_ENG = list("SVSSVSSVSVSSVSVS")


# ---------------------------------------------------------------------------
# BIR wait legalization: this container's walrus accepts at most ONE sync wait
# per instruction (two for EventSemaphore); hoist excess waits onto inserted
# EventSemaphores on the same engine.
# ---------------------------------------------------------------------------


def _legalize_bir_waits(bir_bytes: bytes) -> bytes:
    import json

    m = json.loads(bir_bytes)
    changed = False
    for fn in m["functions"]:
        for blk in fn["blocks"]:
            new_insts = []
            for inst in blk["instructions"]:
                si = inst.get("sync_info")
                waits = list(si.get("on_wait") or []) if si else []
                cap = 2 if inst.get("opcode") == "EventSemaphore" else 1
                if len(waits) > cap:
                    changed = True
                    keep = waits[-cap:]
                    extra = waits[:-cap]
                    idx = 0
                    while extra:
                        chunk, extra = extra[:2], extra[2:]
                        es = {
                            "name": f"{inst['name']}_ws{idx}",
                            "engine": inst["engine"],
                            "opcode": "EventSemaphore",
                            "ins": [],
                            "outs": [],
                            "sync_info": {"on_wait": chunk, "on_update": []},
                        }
                        if "debug" in inst:
                            es["debug"] = inst["debug"]
                        new_insts.append(es)
                        idx += 1
                    si["on_wait"] = keep
                new_insts.append(inst)
            blk["instructions"] = new_insts
    return json.dumps(m).encode() if changed else bir_bytes


# ---------------------------------------------------------------------------
# Device program (identical on all 8 cores; inputs differ per core)
# ---------------------------------------------------------------------------


def build_nc():
    nc = bass.Bass()

    x_in = nc.dram_tensor("x", [C, T], BF16, kind="ExternalInput")
    # [wq|wq] and [wk|wk] duplicated weights, per 128-channel half
    wqq_in = nc.dram_tensor("wqq", [C, 128], BF16, kind="ExternalInput")
    wkk_in = nc.dram_tensor("wkk", [C, 128], BF16, kind="ExternalInput")
    wvT_in = nc.dram_tensor("wvT", [C, CH], BF16, kind="ExternalInput")
    bqq_in = nc.dram_tensor("bqq", [128, 1], F32, kind="ExternalInput")
    bkk_in = nc.dram_tensor("bkk", [128, 1], F32, kind="ExternalInput")
    # proj weights zero-padded to K=128 (rows CH..127 are zero)
    wpT_in = nc.dram_tensor("wpT", [128, C], BF16, kind="ExternalInput")
    oh_in = nc.dram_tensor("oh", [128, 16], F32, kind="ExternalInput")
    ohT_in = nc.dram_tensor("ohT", [16, 128], F32, kind="ExternalInput")
    gnw_in = nc.dram_tensor("gnw", [C, 1], F32, kind="ExternalInput")
    gnb_in = nc.dram_tensor("gnb", [C, 1], F32, kind="ExternalInput")
    out = nc.dram_tensor("out", [C, T], BF16, kind="ExternalOutput")
    # softmax denominators (host divides during the gather)
    rsum_out = nc.dram_tensor("rsum", [1, T], BF16, kind="ExternalOutput")

    with tile.TileContext(nc) as tc:
        with (
            tc.tile_pool(name="const", bufs=1) as const,
            tc.tile_pool(name="xp", bufs=2) as xp,
            tc.tile_pool(name="xnp", bufs=2) as xnp,
            tc.tile_pool(name="qk", bufs=1) as qkp,
            tc.tile_pool(name="gn", bufs=2) as gn,
            tc.tile_pool(name="pp", bufs=8) as ppool,
            tc.tile_pool(name="op", bufs=4) as op,
            tc.tile_pool(name="ps", bufs=3, space="PSUM") as ps,
            tc.tile_pool(name="ph", bufs=2, space="PSUM") as ph,
        ):
            # early exp-table prefetch: the first scalar ACTIVATE loads the
            # exp set into table slot 0 (Sqrt later takes slot 1), so no
            # table load lands on the attention critical path
            warmup_c = const.tile([2, 1], F32, tag="warmup_c")
            nc.vector.memset(warmup_c, 0.0)
            nc.scalar.activation(out=warmup_c, in_=warmup_c, func=AF.Exp)

            # PE HAM warm-up fodder (zeros): the clock gate needs ~3.4us of
            # sustained matmul activity to release 2.4 GHz
            warm_w = const.tile([128, 512], BF16, tag="warm_w")
            nc.vector.memset(warm_w, 0.0)

            # ---- x load first (bf16, halves per tile so stats can overlap) ----
            x_tiles = []
            for i in range(2):
                x_t = xp.tile([128, T], BF16, tag="x", name=f"x{i}")
                for half in range(2):
                    hsl = slice(half * (T // 2), (half + 1) * (T // 2))
                    nc.sync.dma_start(
                        out=x_t[:, hsl], in_=x_in[i * 128 : (i + 1) * 128, hsl]
                    )
                x_tiles.append(x_t)

            # warm-up burst: dummy matmuls gated on x tile 1's arrival so the
            # PE clock is warm when the first score pairs issue (~10us later)
            wps = ps.tile([128, 1024], F32, tag="ps", name="warm_ps")
            for i in range(20):
                nc.tensor.matmul(
                    wps[:, 0:512],
                    lhsT=warm_w[:, 0:128],
                    rhs=x_tiles[1][:, 0:512],
                    start=True, stop=True,
                )

            # ---- load constants/weights ----
            def load_const(name, src, shape, dtype):
                t = const.tile(shape, dtype, tag=name)
                nc.gpsimd.dma_start(out=t, in_=src[:, :])
                return t

            wqq = [
                const.tile([128, 128], BF16, tag=f"wqq{k}", name=f"wqq{k}")
                for k in range(2)
            ]
            wkk = [
                const.tile([128, 128], BF16, tag=f"wkk{k}", name=f"wkk{k}")
                for k in range(2)
            ]
            wv = [
                const.tile([128, CH], BF16, tag=f"wv{k}", name=f"wv{k}")
                for k in range(2)
            ]
            for k in range(2):
                sl = slice(k * 128, (k + 1) * 128)
                nc.gpsimd.dma_start(out=wqq[k], in_=wqq_in[sl, :])
                nc.gpsimd.dma_start(out=wkk[k], in_=wkk_in[sl, :])
                nc.gpsimd.dma_start(out=wv[k], in_=wvT_in[sl, :])

            bqq_sb = load_const("bqq", bqq_in, [128, 1], F32)
            bkk_sb = load_const("bkk", bkk_in, [128, 1], F32)
            wpT_sb = load_const("wpT", wpT_in, [128, C], BF16)
            oh_sb = load_const("oh", oh_in, [128, 16], F32)
            ohT_sb = load_const("ohT", ohT_in, [16, 128], F32)

            gnw_t = [
                const.tile([128, 1], F32, tag=f"gnw{i}", name=f"gnw{i}")
                for i in range(2)
            ]
            gnb_t = [
                const.tile([128, 1], F32, tag=f"gnb{i}", name=f"gnb{i}")
                for i in range(2)
            ]
            for i in range(2):
                nc.gpsimd.dma_start(out=gnw_t[i], in_=gnw_in[i * 128 : (i + 1) * 128, :])
                nc.gpsimd.dma_start(out=gnb_t[i], in_=gnb_in[i * 128 : (i + 1) * 128, :])

            eps_t = const.tile([16, 1], F32, tag="eps")
            nc.vector.memset(eps_t, EPS)
            nln2_t = const.tile([128, 1], F32, tag="nln2")
            nc.vector.memset(nln2_t, -LN2)

            # persistent hu tiles: rows 65..127 zeroed once (row 64 = denom,
            # masked by wpT's zero rows)
            hu_t = []
            for i in range(2):
                t = const.tile([128, TC], BF16, tag=f"hu{i}", name=f"hu{i}")
                nc.vector.memset(t[CH:128, :], 0.0)
                hu_t.append(t)

            # vT with ones columns (fp8): [128 keys, 32 blocks, 80] -- cols
            # 0:64 v channels, col 64 ones (denominator row of ps_h)
            vT = qkp.tile([128, 32 * 80], FP8, tag="vT")
            nc.gpsimd.memset(vT, 1.0)
            vT_view = vT.rearrange("p (b c) -> p b c", c=80)

            # ---- GroupNorm stats -> per-channel affine (a_ch, b_ch) ----
            ab_ch = []  # per tile: (a, b)
            for i in range(2):
                x_t = x_tiles[i]
                xv = x_t.rearrange("p (n f) -> p n f", f=512)
                stats = gn.tile([128, 8, 6], F32, tag="stats")
                for j in range(8):
                    nc.vector.bn_stats(out=stats[:, j, :], in_=xv[:, j, :])
                    if j % 2 == 1:
                        wph = ph.tile([6, 16], F32, tag="ph", name=f"wt{i}_{j}")
                        nc.tensor.matmul(
                            wph[0:6, 0:6], lhsT=stats[:, j, :], rhs=stats[:, j, :],
                            start=True, stop=True,
                        )
                mv = gn.tile([128, 2], F32, tag="mv")
                nc.vector.bn_aggr(out=mv, in_=stats)

                # mq = [mean, var + mean^2] per channel
                mq = gn.tile([128, 2], F32, tag="mq")
                nc.vector.tensor_copy(out=mq[:, 0:1], in_=mv[:, 0:1])
                m2 = gn.tile([128, 1], F32, tag="m2")
                nc.vector.tensor_tensor(
                    out=m2, in0=mv[:, 0:1], in1=mv[:, 0:1], op=ALU.mult
                )
                nc.vector.tensor_tensor(
                    out=mq[:, 1:2], in0=mv[:, 1:2], in1=m2, op=ALU.add
                )

                # group reduce: [16, 2] = oh.T @ mq   (oh entries are 1/8)
                ps_g = ph.tile([16, 2], F32, tag="ph")
                nc.tensor.matmul(ps_g, lhsT=oh_sb, rhs=mq, start=True, stop=True)
                gstats = gn.tile([16, 2], F32, tag="gstats")
                nc.vector.tensor_copy(out=gstats, in_=ps_g)

                gm2 = gn.tile([16, 1], F32, tag="gm2")
                nc.vector.tensor_tensor(
                    out=gm2, in0=gstats[:, 0:1], in1=gstats[:, 0:1], op=ALU.mult
                )
                gvar = gn.tile([16, 1], F32, tag="gvar")
                nc.vector.tensor_tensor(
                    out=gvar, in0=gstats[:, 1:2], in1=gm2, op=ALU.subtract
                )
                sq = gn.tile([16, 1], F32, tag="sq")
                nc.scalar.activation(out=sq, in_=gvar, func=AF.Sqrt, bias=eps_t)
                grstd = gn.tile([16, 1], F32, tag="grstd")
                nc.vector.reciprocal(out=grstd, in_=sq)
                gmr = gn.tile([16, 2], F32, tag="gmr")
                nc.vector.tensor_copy(out=gmr[:, 0:1], in_=gstats[:, 0:1])
                nc.vector.tensor_copy(out=gmr[:, 1:2], in_=grstd)

                # broadcast back to channels: [128, 2] = ohT.T @ gmr
                ps_bc = ph.tile([128, 2], F32, tag="ph")
                nc.tensor.matmul(ps_bc, lhsT=ohT_sb, rhs=gmr, start=True, stop=True)

                a_ch = gn.tile([128, 1], F32, tag="a_ch")
                nc.vector.tensor_tensor(
                    out=a_ch, in0=ps_bc[:, 1:2], in1=gnw_t[i], op=ALU.mult
                )
                t1 = gn.tile([128, 1], F32, tag="t1")
                nc.vector.tensor_tensor(
                    out=t1, in0=ps_bc[:, 0:1], in1=a_ch, op=ALU.mult
                )
                b_ch = gn.tile([128, 1], F32, tag="b_ch")
                nc.vector.tensor_tensor(
                    out=b_ch, in0=gnb_t[i], in1=t1, op=ALU.subtract
                )
                ab_ch.append((a_ch, b_ch))

            # ---- xn tiles (bf16), emitted per 1024-col chunk on DVE ----
            xn_tiles = []
            for i in range(2):
                xn_t = xnp.tile([128, T], BF16, tag="xn", name=f"xn{i}")
                xn_tiles.append(xn_t)

            def emit_xn_chunk(c):
                csl = slice(c * 1024, (c + 1) * 1024)
                for i in range(2):
                    a_ch, b_ch = ab_ch[i]
                    with nc.allow_low_precision(reason="bf16 xn"):
                        nc.vector.tensor_scalar(
                            out=xn_tiles[i][:, csl],
                            in0=x_tiles[i][:, csl],
                            scalar1=a_ch,
                            scalar2=b_ch,
                            op0=ALU.mult,
                            op1=ALU.add,
                        )

            # ---- q2 (dup) / k2 (even-odd) / vT emission ----
            q2 = qkp.tile([128, T], BF16, tag="q2")
            k2 = qkp.tile([128, 2048], BF16, tag="k2")

            def emit_q_chunk(c):
                csl = slice(c * 1024, (c + 1) * 1024)
                psq = ps.tile([128, 1024], F32, tag="ps", name=f"psq{c}")
                for nj in range(2):
                    sl = slice(nj * 512, (nj + 1) * 512)
                    xsl = slice(c * 1024 + nj * 512, c * 1024 + (nj + 1) * 512)
                    for ki in range(2):
                        nc.tensor.matmul(
                            psq[:, sl],
                            lhsT=wqq[ki],
                            rhs=xn_tiles[ki][:, xsl],
                            start=(ki == 0),
                            stop=(ki == 1),
                        )
                with nc.allow_low_precision(reason="bf16 q"):
                    if c % 2 == 0:
                        nc.scalar.activation(
                            out=q2[:, csl], in_=psq, func=AF.Identity, bias=bqq_sb
                        )
                    else:
                        nc.vector.tensor_scalar(
                            out=q2[:, csl], in0=psq, scalar1=bqq_sb,
                            scalar2=None, op0=ALU.add,
                        )

            def emit_k_chunk(c):
                # chunk c covers blocks 8c..8c+7 -> k2 cols [c*512,(c+1)*512)
                psk = ps.tile([128, 1024], F32, tag="ps", name=f"psk{c}")
                for nj in range(2):
                    sl = slice(nj * 512, (nj + 1) * 512)
                    xsl = slice(c * 1024 + nj * 512, c * 1024 + (nj + 1) * 512)
                    for ki in range(2):
                        nc.tensor.matmul(
                            psk[:, sl],
                            lhsT=wkk[ki],
                            rhs=xn_tiles[ki][:, xsl],
                            start=(ki == 0),
                            stop=(ki == 1),
                        )
                ksl = slice(c * 512, (c + 1) * 512)
                pv = psk.rearrange("p (m t c) -> p m t c", t=2, c=128)
                with nc.allow_low_precision(reason="bf16 k"):
                    # even blocks -> rows 0:64, odd -> rows 64:128
                    nc.vector.tensor_scalar(
                        out=k2[0:64, ksl].rearrange("p (m c) -> p m c", c=128),
                        in0=pv[0:64, :, 0:1, :].rearrange("p m t c -> p m (t c)"),
                        scalar1=bkk_sb[0:64],
                        scalar2=None,
                        op0=ALU.add,
                    )
                    nc.vector.tensor_scalar(
                        out=k2[64:128, ksl].rearrange("p (m c) -> p m c", c=128),
                        in0=pv[64:128, :, 1:2, :].rearrange("p m t c -> p m (t c)"),
                        scalar1=bkk_sb[64:128],
                        scalar2=None,
                        op0=ALU.add,
                    )

            def emit_v_chunk(c):
                # blocks 8c..8c+7
                psv = ph.tile([128, 512], F32, tag="ph", name=f"psv{c}")
                for j in range(8):
                    sblk = c * 8 + j
                    sl = slice(j * 64, (j + 1) * 64)
                    for ki in range(2):
                        nc.tensor.matmul(
                            psv[:, sl],
                            lhsT=xn_tiles[ki][:, sblk * 128 : (sblk + 1) * 128],
                            rhs=wv[ki],
                            start=(ki == 0),
                            stop=(ki == 1),
                        )
                with nc.allow_low_precision(reason="fp8 v"):
                    nc.scalar.copy(
                        out=vT_view[:, c * 8 : (c + 1) * 8, 0:64],
                        in_=psv.rearrange("p (b c) -> p b c", c=64),
                    )

            emit_xn_chunk(0)
            emit_q_chunk(0)
            emit_k_chunk(0)
            emit_v_chunk(0)
            for c in range(1, 4):
                emit_xn_chunk(c)

            # ---- attention + proj, streamed per t-chunk ----
            def body(tci, carry=None):
                tsl = slice(tci * TC, (tci + 1) * TC)
                ps_h = ph.tile([65, TC], F32, tag="ph", name=f"ps_h{tci}")
                pend_dr = []

                def flush_drs():
                    for pr, m in pend_dr:
                        nc.tensor.matmul(
                            ps_h,
                            lhsT=vT_view[:, 2 * m : 2 * m + 2, 0:65],
                            rhs=pr,
                            start=(m == 0),
                            stop=(m == N_PAIRS - 1),
                            perf_mode=DRMODE,
                        )
                    pend_dr.clear()

                def flush_carry():
                    if carry is None:
                        return
                    c_ps_h, c_pend = carry
                    for pr, m in c_pend:
                        nc.tensor.matmul(
                            c_ps_h,
                            lhsT=vT_view[:, 2 * m : 2 * m + 2, 0:65],
                            rhs=pr,
                            start=(m == 0),
                            stop=(m == N_PAIRS - 1),
                            perf_mode=DRMODE,
                        )
                    c_pend.clear()

                for m in range(N_PAIRS):
                    # JIT emission: k/v chunks during tchunk 0, q chunks on
                    # odd tchunks (q chunk c gates tchunk 2c)
                    if tci == 0 and m in (4, 8, 12):
                        flush_drs()
                        emit_k_chunk(m // 4)
                    if tci == 0 and m in (6, 10, 14):
                        emit_v_chunk(m // 4)
                    if tci in (1, 3, 5) and m == 4:
                        flush_drs()
                        emit_q_chunk((tci + 1) // 2)
                    ps_s = ps.tile([128, 1024], F32, tag="ps", name=f"ps_s{tci}_{m}")
                    # row-tiled pair: even block (rows 0:64) + odd (64:128)
                    nc.tensor.matmul(
                        ps_s[:, 0:512],
                        lhsT=k2[0:64, m * 128 : (m + 1) * 128],
                        rhs=q2[0:64, tsl],
                        start=True,
                        stop=True,
                    )
                    nc.tensor.matmul(
                        ps_s[:, 512:1024],
                        lhsT=k2[64:128, m * 128 : (m + 1) * 128],
                        rhs=q2[64:128, tsl],
                        start=True,
                        stop=True,
                    )
                    p_t = ppool.tile([128, 1024], FP8, tag="p", name=f"p{tci}_{m}")
                    with nc.allow_low_precision(reason="fp8 p"):
                        if EXP_ENG[m] == "S":
                            nc.scalar.activation(
                                out=p_t,
                                in_=ps_s,
                                func=AF.Exp,
                                scale=0.125,
                                bias=nln2_t,
                            )
                        else:
                            nc.vector.tensor_scalar(
                                out=p_t.bitcast(U8),
                                in0=ps_s,
                                scalar1=SCH_A,
                                scalar2=SCH_B,
                                op0=ALU.mult,
                                op1=ALU.add,
                            )
                    pend_dr.append((p_t.rearrange("p (b c) -> p b c", c=TC), m))
                    if m % 4 == 3 and m < N_PAIRS - 1:
                        flush_drs()
                return ps_h, pend_dr

            def epilogue(tci, ps_h):
                tsl = slice(tci * TC, (tci + 1) * TC)
                hu = hu_t[tci % 2]
                with nc.allow_low_precision(reason="bf16 h"):
                    nc.vector.tensor_copy(out=hu[0:65, :], in_=ps_h[0:65, :])
                nc.sync.dma_start(out=rsum_out[0:1, tsl], in_=hu[64:65, :])
                for mi in range(2):
                    pp_ps = ph.tile([128, TC], F32, tag="ph", name=f"pp{tci}_{mi}")
                    nc.tensor.matmul(
                        pp_ps,
                        lhsT=wpT_sb[:, mi * 128 : (mi + 1) * 128],
                        rhs=hu,
                        start=True,
                        stop=True,
                    )
                    o_t = op.tile([128, TC], BF16, tag="o", name=f"o{tci}_{mi}")
                    with nc.allow_low_precision(reason="bf16 out"):
                        nc.vector.tensor_copy(out=o_t, in_=pp_ps)
                    nc.sync.dma_start(
                        out=out[mi * 128 : (mi + 1) * 128, tsl], in_=o_t
                    )

            # software pipeline: the last DR batch of chunk i is flushed
            # early in chunk i+1, and epilogue(i-1) is emitted after body(i)
            prev = None
            for tci in range(N_TCHUNKS):
                cur = body(tci, carry=prev)
                if prev is not None:
                    epilogue(tci - 1, prev[0])
                prev = cur
            # drain the final tchunk
            ps_h_f, pend_f = prev
            for pr, m in pend_f:
                nc.tensor.matmul(
                    ps_h_f,
                    lhsT=vT_view[:, 2 * m : 2 * m + 2, 0:65],
                    rhs=pr,
                    start=(m == 0),
                    stop=(m == N_PAIRS - 1),
                    perf_mode=DRMODE,
                )
            epilogue(N_TCHUNKS - 1, ps_h_f)

    orig = nc.to_json_bytes
    nc.to_json_bytes = lambda *a, **k: _legalize_bir_waits(orig(*a, **k))
    return nc


_NC = None


def _get_nc():
    global _NC
    if _NC is None:
        _NC = build_nc()
    return _NC


def _make_in_maps(inputs):
    x = np.asarray(inputs["x"], dtype=np.float32)
    gn_w = np.asarray(inputs["gn_w"], dtype=np.float32)
    gn_b = np.asarray(inputs["gn_b"], dtype=np.float32)
    qkv_w = np.asarray(inputs["qkv_w"], dtype=np.float32)
    qkv_b = np.asarray(inputs["qkv_b"], dtype=np.float32)
    proj_w = np.asarray(inputs["proj_w"], dtype=np.float32)

    xs = x.reshape(B, C, T).astype(ml_dtypes.bfloat16)
    oh = np.kron(np.eye(16, dtype=np.float32), np.full((8, 1), 0.125, np.float32))
    ohT = np.ascontiguousarray(oh.T) * 8.0  # plain one-hot [16, 128]
    gnw = gn_w.reshape(C, 1)
    gnb = gn_b.reshape(C, 1)

    in_maps = []
    for core in range(N_CORES):
        b, h = divmod(core, NH)
        # reference reshapes (b, 3c, T) -> (b*nh, 3*ch, T) then splits dim 1,
        # so head h takes qkv rows [3*ch*h : 3*ch*(h+1)] as [q | k | v]
        base = 3 * CH * h
        qsl = slice(base, base + CH)
        ksl = slice(base + CH, base + 2 * CH)
        vsl = slice(base + 2 * CH, base + 3 * CH)
        wqT = np.ascontiguousarray(qkv_w[qsl, :].T)  # [C, CH]
        wkT = np.ascontiguousarray(qkv_w[ksl, :].T)
        wqq = np.concatenate([wqT, wqT], axis=1).astype(ml_dtypes.bfloat16)
        wkk = np.concatenate([wkT, wkT], axis=1).astype(ml_dtypes.bfloat16)
        wvT = np.ascontiguousarray(qkv_w[vsl, :].T).astype(ml_dtypes.bfloat16)
        bq = qkv_b[qsl].astype(np.float32)
        bk = qkv_b[ksl].astype(np.float32)
        bqq = np.concatenate([bq, bq]).reshape(128, 1)
        bkk = np.concatenate([bk, bk]).reshape(128, 1)
        # after attention, head h occupies channels [ch*h : ch*(h+1)]
        wpT = np.zeros((128, C), ml_dtypes.bfloat16)
        wpT[0:CH] = proj_w[:, h * CH : (h + 1) * CH].T.astype(ml_dtypes.bfloat16)
        in_maps.append(
            {
                "x": np.ascontiguousarray(xs[b]),
                "wqq": wqq,
                "wkk": wkk,
                "wvT": wvT,
                "bqq": bqq,
                "bkk": bkk,
                "wpT": wpT,
                "oh": oh,
                "ohT": ohT,
                "gnw": gnw,
                "gnb": gnb,
            }
        )
    return in_maps


def _combine(inputs, results):
    x = np.asarray(inputs["x"], dtype=np.float32)
    proj_b = np.asarray(inputs["proj_b"], dtype=np.float32)
    qkv_b = np.asarray(inputs["qkv_b"], dtype=np.float32)
    proj_w = np.asarray(inputs["proj_w"], dtype=np.float32)
    xs = x.reshape(B, C, T)
    out = np.empty((B, C, T), np.float32)
    for b in range(B):
        acc = xs[b] + proj_b[:, None]
        for h in range(NH):
            r = results[b * NH + h]
            # v's bias bv contributes bv (x) rowsum to the unnormalized h;
            # after proj and the rowsum division it is the constant vector
            # proj_w[:, head] @ bv -- folded here instead of on device
            bv = qkv_b[3 * CH * h + 2 * CH : 3 * CH * (h + 1)]
            wpbv = proj_w[:, h * CH : (h + 1) * CH] @ bv
            acc = (
                acc
                + r["out"].astype(np.float32)
                * (1.0 / r["rsum"][0].astype(np.float32))[None, :]
                + wpbv[:, None]
            )
        out[b] = acc
    return out.reshape(B, C, HW, HW)


def _run(inputs, trace=False, trace_kwargs=None):
    nc = _get_nc()
    in_maps = _make_in_maps(inputs)
    res = run_bass_kernel_spmd(
        nc,
        in_maps,
        core_ids=list(range(N_CORES)),
        trace=trace,
        **(trace_kwargs or {}),
    )
    return _combine(inputs, res.results), res


def kernel(**inputs) -> np.ndarray:
    out, _ = _run(inputs, trace=False)
    return out
